# revision 1
# baseline (speedup 1.0000x reference)
"""Kernelized (linear) attention on 8 TRN2 NeuronCores — v11 (staged 2-way otl gather).

vs v1 baseline (same math, same T-data-parallel main loop):
  - Wk/Wv arrive SHARDED (2 of 16 output-tiles per core, 2MB vs 16MB) and
    are AllGather'd on-device; host->device bytes drop ~2.5x overall
    (404MB -> ~162MB across the 8 cores).
  - Stats use ReduceScatter (transposed layout) instead of AllReduce, so
    each core receives exactly its own 2 heads' stats at fixed offsets.
  - Final stage (q proj, combine, Wo) is tensor-parallel: each core only
    computes its own 2 heads with its Wq/Wo shard (1MB+1MB vs 8MB+8MB),
    followed by a 32KB AllReduce of the partial [B,D] output (the
    "row-split W_o + one all-reduce" from the sharding hint).
  - Matmuls use 1024-col bf16 moving operands (half the instructions),
    stationary weight tile shared across both r-chunks.

Layout per core c (hardcoded for D=2048, H=16, T=4096, B=4, 8 cores):
  xk/xv: [D, R=2048] bf16, columns r = b*512 + t_local, t in [c*512,(c+1)*512)
  wkv:   [512, D] bf16 rows = [kv(2), otl(2), p(128)]; block ot = 2c+otl in
         wblocks layout [p = d%128, (d//128)*128 + o_in]
  wq:    [2, 128, D] bf16 wblocks layout for ot = 2c+otl
  wo:    [2, 128, D] bf16 = Wo.T[ot block rows, :] for ot = 2c+otl
  qt:    [128, NT*B] bf16, [p = d%128, (d//128)*B + b]
  Stats column index (before transpose+ReduceScatter):
    kv(ot,b) -> (ot//2)*16 + (ot%2)*4 + b ;  ks(ot,b) -> same + 8
  so RS chunk c = rows [16c, 16c+16) of the transposed [128,128] stats.
"""

import numpy as np
import ml_dtypes

from concourse import bass, bacc, mybir, tile
from concourse.bass_utils import run_bass_kernel_spmd

BF16 = ml_dtypes.bfloat16

D, H, T, B = 2048, 16, 4096, 4
HD = D // H           # 128
NCORES = 8
TLOC = T // NCORES    # 512 history rows per core
R = TLOC * B          # 2048 projection rows per core
NT = D // 128         # 16 tiles along d (contraction) and o (output)
OTC = NT // NCORES    # 2 output tiles (heads) owned per core
EPS = 1e-6
F32 = mybir.dt.float32
BF = mybir.dt.bfloat16
F8 = mybir.dt.float8e4
WSC = 64.0    # host pre-scale on Wk/Wv/Wq/Wo so fp8 stays in normal range
OSC = 8192.0  # on-chip pre-scale on opre (tiny values) before fp8 cast
AF = mybir.ActivationFunctionType
OP = mybir.AluOpType


def build_nc():
    nc = bacc.Bacc("TRN2", target_bir_lowering=False, debug=False,
                   enable_asserts=False, num_devices=NCORES)

    def din(name, shape, dt):
        return nc.dram_tensor(name, list(shape), dt, kind="ExternalInput").ap()

    xk_d = din("xk", (D, R), F8)
    xv_d = din("xv", (D, R), F8)
    wkv_d = din("wkv", (2 * OTC * 128, D), F8)
    wq_d = din("wq", (OTC, 128, D), F8)
    wo_d = din("wo", (OTC, 128, D), F8)
    qt_d = din("qt", (128, NT * B), F8)
    bk_d = din("bk", (128, NT), F32)
    bv8_d = din("bv8", (128, OTC * B), F32)
    bq_d = din("bq", (128, OTC), F32)
    bo8_d = din("bo8", (B, D), F32)       # bo / NCORES, tiled over 4 rows
    al_d = din("al", (1, OTC * B), F32)
    eye_d = din("eye", (128, 128), F32)
    onc_d = din("onc", (128, 1), F32)
    onr_d = din("onr", (1, 128), F32)

    # [128, 64] row-major == [B, D] row-major flat; reshaped host-side
    out_d = nc.dram_tensor("out", [128, (B * D) // 128], F32,
                           kind="ExternalOutput").ap()

    with tile.TileContext(nc) as tc:
        import contextlib
        with contextlib.ExitStack() as ctx:
            p_xk = ctx.enter_context(tc.tile_pool(name="xk", bufs=NT))
            p_xv = ctx.enter_context(tc.tile_pool(name="xv", bufs=NT))
            p_w = ctx.enter_context(tc.tile_pool(name="w", bufs=2))
            p_ep = ctx.enter_context(tc.tile_pool(name="ep", bufs=3))
            p_pr = ctx.enter_context(tc.tile_pool(name="pr", bufs=2))
            p_c1 = ctx.enter_context(tc.tile_pool(name="c1", bufs=1))
            p_qk = ctx.enter_context(tc.tile_pool(name="qk", bufs=2))
            p_st = ctx.enter_context(tc.tile_pool(name="st", bufs=2))
            p_dr = ctx.enter_context(tc.tile_pool(name="dr", bufs=1, space="DRAM"))

            # ---- stage + AllGather the Wk/Wv shards in two otl halves
            # (host layout is otl-major: rows [k-otl0, v-otl0, k-otl1,
            # v-otl1] x 128); collectives cannot read IO tensors, so each
            # half bounces through internal DRAM. The otl-major main loop
            # starts once the first half lands.
            wg_outs = []
            for h in range(OTC):
                wg_in = p_dr.tile([256, D], F8, tag=f"wgin{h}",
                                  name=f"wg_in{h}")
                for i in range(2):
                    r0 = h * 256 + i * 128
                    wtmp = p_st.tile([128, D], F8, tag="wst")
                    nc.sync.dma_start(out=wtmp[:], in_=wkv_d[r0:r0 + 128, :])
                    nc.sync.dma_start(out=wg_in[i * 128:(i + 1) * 128, :],
                                      in_=wtmp[:])
                wg_out = p_dr.tile([NCORES * 256, D], F8, tag=f"wgout{h}",
                                   name=f"wg_out{h}", addr_space="Shared")
                nc.gpsimd.collective_compute(
                    "AllGather", OP.bypass,
                    replica_groups=[list(range(NCORES))],
                    ins=[wg_in.opt()], outs=[wg_out.opt()])
                wg_outs.append(wg_out)

            # ---- resident loads -------------------------------------------
            xk_t, xv_t = [], []
            for tt in range(NT // 2):
                a = p_xk.tile([128, 2, R], F8, tag="xk")
                for s in range(2):
                    r0 = (2 * tt + s) * 128
                    nc.sync.dma_start(out=a[:, s], in_=xk_d[r0:r0 + 128, :])
                xk_t.append(a)
                b_ = p_xv.tile([128, 2, R], F8, tag="xv")
                for s in range(2):
                    r0 = (2 * tt + s) * 128
                    nc.sync.dma_start(out=b_[:, s], in_=xv_d[r0:r0 + 128, :])
                xv_t.append(b_)

            bk_s = p_c1.tile([128, NT], F32, tag="bk")
            nc.sync.dma_start(out=bk_s[:], in_=bk_d[:, :])
            bv8_s = p_c1.tile([128, OTC * B], F32, tag="bv8")
            nc.sync.dma_start(out=bv8_s[:], in_=bv8_d[:, :])
            bq_s = p_c1.tile([128, OTC], F32, tag="bq")
            nc.sync.dma_start(out=bq_s[:], in_=bq_d[:, :])
            bo8_s = p_c1.tile([B, D], F32, tag="bo8")
            nc.sync.dma_start(out=bo8_s[:], in_=bo8_d[:, :])
            al_s = p_c1.tile([1, OTC * B], F32, tag="al")
            nc.sync.dma_start(out=al_s[:], in_=al_d[:, :])
            eye_s = p_c1.tile([128, 128], F32, tag="eye")
            nc.sync.dma_start(out=eye_s[:], in_=eye_d[:, :])
            onc_s = p_c1.tile([128, 1], F32, tag="onc")
            nc.sync.dma_start(out=onc_s[:], in_=onc_d[:, :])
            onr_s = p_c1.tile([1, 128], F32, tag="onr")
            nc.sync.dma_start(out=onr_s[:], in_=onr_d[:, :])
            qt_s = p_c1.tile([128, NT * B], F8, tag="qt")
            nc.sync.dma_start(out=qt_s[:], in_=qt_d[:, :])
            wq_s, wo_sl = [], []
            for ol in range(OTC):
                w = p_c1.tile([128, D], F8, tag=f"wq{ol}")
                nc.sync.dma_start(out=w[:], in_=wq_d[ol])
                wq_s.append(w)
                wo_ = p_c1.tile([128, D], F8, tag=f"wo{ol}")
                nc.sync.dma_start(out=wo_[:], in_=wo_d[ol])
                wo_sl.append(wo_)

            # stats, transposed-RS column order (see module docstring)
            stat = p_c1.tile([128, 2 * H * B], F32, tag="stat")

            def kv_col(ot, b):
                return (ot // OTC) * 16 + (ot % OTC) * 4 + b

            # ---- main loop: K/V projections + fused stats -----------------
            with tc.tile_pool(name="mmk", bufs=3, space="PSUM") as p_mk, \
                 tc.tile_pool(name="mmv", bufs=3, space="PSUM") as p_mv:
                NTT = NT // 2
                DR = mybir.MatmulPerfMode.DoubleRow
                for otl in range(OTC):
                  for csrc in range(NCORES):
                    ot = csrc * OTC + otl
                    base = csrc * 256
                    wk_s = p_w.tile([128, NTT, 2, 128], F8, tag="wk")
                    nc.sync.dma_start(
                        out=wk_s[:],
                        in_=wg_outs[otl][base:base + 128, :])
                    wv_s = p_w.tile([128, NTT, 2, 128], F8, tag="wv")
                    nc.sync.dma_start(
                        out=wv_s[:],
                        in_=wg_outs[otl][base + 128:base + 256, :])
                    for b in range(B):           # 512-col (per-batch) chunks
                        c0 = b * 512
                        ck = kv_col(ot, b)
                        kp = p_mk.tile([128, 512], F32, tag="mmk")
                        for tt in range(NTT):
                            nc.tensor.matmul(
                                kp[:], wk_s[:, tt],
                                xk_t[tt][:, :, c0:c0 + 512],
                                start=(tt == 0), stop=(tt == NTT - 1),
                                perf_mode=DR)
                        vp = p_mv.tile([128, 512], F32, tag="mmv")
                        for tt in range(NTT):
                            nc.tensor.matmul(
                                vp[:], wv_s[:, tt],
                                xv_t[tt][:, :, c0:c0 + 512],
                                start=(tt == 0), stop=(tt == NTT - 1),
                                perf_mode=DR)
                        kk = p_ep.tile([128, 512], F32, tag="kk")
                        nc.scalar.activation(
                            kk[:], kp[:], AF.Relu,
                            bias=bk_s[:, ot:ot + 1], scale=1.0 / WSC,
                            accum_out=stat[:, 8 + ck:8 + ck + 1])
                        pr = p_pr.tile([128, 512], BF, tag="pr")
                        nc.vector.scalar_tensor_tensor(
                            pr[:], kk[:], EPS, vp[:], OP.add, OP.mult,
                            accum_out=stat[:, ck:ck + 1])

            # NOTE on stat columns: kv_col gives the within-chunk offsets
            # 0..7 (kv) and 8..15 (ks) for chunk csrc; chunk base is 16*csrc.
            # kv at 16*csrc + otl*4 + b  = kv_col(ot,b)
            # ks at 16*csrc + 8 + otl*4 + b = kv_col(ot,b) + 8
            # (the expressions above index stat accordingly)

            with tc.tile_pool(name="fin", bufs=2, space="PSUM") as p_fin, \
                 tc.tile_pool(name="tn", bufs=2, space="PSUM") as p_tn, \
                 tc.tile_pool(name="opp", bufs=1, space="PSUM") as p_op:
                # ---- transpose stats, ReduceScatter ------------------------
                stT_ps = p_tn.tile([128, 128], F32, tag="tn", name="stTp")
                nc.tensor.transpose(stT_ps[:], stat[:], eye_s[:])
                stT = p_st.tile([128, 128], F32, tag="stT")
                nc.vector.tensor_copy(stT[:], stT_ps[:])
                rs_in = p_dr.tile([128, 128], F32, tag="rsin")
                nc.gpsimd.dma_start(out=rs_in[:], in_=stT[:])
                rs_out = p_dr.tile([16, 128], F32, tag="rsout")
                nc.gpsimd.collective_compute(
                    "ReduceScatter", OP.add,
                    replica_groups=[list(range(NCORES))],
                    ins=[rs_in.opt()], outs=[rs_out.opt()])

                # ---- q projection for own 2 output tiles (overlaps RS) ----
                qps = []
                for ol in range(OTC):
                    qp = p_fin.tile([128, B], F32, tag="fin", name=f"qp{ol}")
                    for t in range(NT):
                        nc.tensor.matmul(
                            qp[:], wq_s[ol][:, t * 128:(t + 1) * 128],
                            qt_s[:, t * B:(t + 1) * B],
                            start=(t == 0), stop=(t == NT - 1))
                    qps.append(qp)

                # own stats back to [128 channels, 16]: cols 0:8 kv, 8:16 ks
                rsb = p_st.tile([16, 128], F32, tag="rsb")
                nc.gpsimd.dma_start(out=rsb[:], in_=rs_out[:])
                own_ps = p_tn.tile([128, 16], F32, tag="tn", name="ownp")
                nc.tensor.transpose(own_ps[:], rsb[:], eye_s[:16, :16])
                own = p_st.tile([128, 16], F32, tag="own")
                nc.vector.tensor_copy(own[:], own_ps[:])
                kvb = p_st.tile([128, OTC * B], F32, tag="kvb")
                nc.vector.scalar_tensor_tensor(
                    kvb[:], own[:, 8:16], T * EPS, bv8_s[:], OP.add, OP.mult)
                kvc = p_st.tile([128, OTC * B], F32, tag="kvc")
                nc.vector.scalar_tensor_tensor(
                    kvc[:], own[:, 0:8], 1.0 / WSC, kvb[:], OP.mult, OP.add)

                # ---- combine stats for own 2 heads -------------------------
                hs = p_tn.tile([1, OTC * B], F32, tag="tn", name="hs")
                nc.tensor.matmul(hs[:], onc_s[:], own[:, 8:16],
                                 start=True, stop=True)
                den = p_c1.tile([1, OTC * B], F32, tag="den")
                nc.vector.tensor_scalar(den[:], hs[:], EPS * T * HD + EPS,
                                        None, OP.add)
                rden = p_c1.tile([1, OTC * B], F32, tag="rden")
                nc.vector.reciprocal(rden[:], den[:])
                # al carries alpha*OSC (host-folded opre fp8 pre-scale)
                rr = p_c1.tile([1, OTC * B], F32, tag="rr")
                nc.vector.tensor_tensor(rr[:], rden[:], al_s[:], OP.mult)
                bcr = p_tn.tile([128, OTC * B], F32, tag="tn", name="bcr")
                nc.tensor.matmul(bcr[:], onr_s[:], rr[:], start=True,
                                 stop=True)
                kvr = p_c1.tile([128, OTC * B], F32, tag="kvr")
                nc.vector.tensor_tensor(kvr[:], kvc[:], bcr[:], OP.mult)

                # ---- own-head epilogue + row-split Wo ----------------------
                op_ps = p_op.tile([B, D], F32, tag="opp")
                for ol in range(OTC):
                    qkt = p_qk.tile([128, B], F32, tag="qkt")
                    nc.scalar.activation(qkt[:], qps[ol][:], AF.Relu,
                                         bias=bq_s[:, ol:ol + 1],
                                         scale=1.0 / WSC)
                    opre = p_qk.tile([128, B], F8, tag="opre")
                    nc.vector.scalar_tensor_tensor(
                        opre[:], qkt[:], EPS,
                        kvr[:, ol * B:(ol + 1) * B], OP.add, OP.mult)
                    wo_s = wo_sl[ol]
                    for hh in range(4):
                        nc.tensor.matmul(
                            op_ps[:, hh * 512:(hh + 1) * 512], opre[:],
                            wo_s[:, hh * 512:(hh + 1) * 512],
                            start=(ol == 0), stop=(ol == OTC - 1))

                # un-scale (opre*OSC @ wo*WSC), fold in bo/8 per core
                opart = p_c1.tile([B, D], F32, tag="opart")
                nc.vector.scalar_tensor_tensor(
                    opart[:], op_ps[:], 1.0 / (OSC * WSC), bo8_s[:],
                    OP.mult, OP.add)

            # ---- all-reduce partial outputs -------------------------------
            or_in = p_dr.tile([B, D], F32, tag="orin")
            or_out = p_dr.tile([128, (B * D) // 128], F32, tag="orout",
                               addr_space="Shared")
            nc.gpsimd.dma_start(out=or_in[:], in_=opart[:])
            nc.gpsimd.collective_compute(
                "AllReduce", OP.add,
                replica_groups=[list(range(NCORES))],
                ins=[or_in.opt()], outs=[or_out.opt()])
            osum = p_c1.tile([128, (B * D) // 128], F32, tag="osum")
            nc.gpsimd.dma_start(out=osum[:], in_=or_out[:])
            nc.sync.dma_start(out=out_d[:, :], in_=osum[:])

    nc.finalize()
    from concourse import bass_interp
    nc.m = bass_interp.get_hw_module(nc.m)
    return nc


def prep_inputs(q, k_history, v_history, Wq, bq, Wk, bk, Wv, bv, Wo, bo, alpha):
    """Host-side sharding + layout transforms. Returns in_maps for 8 cores."""
    f32 = np.float32

    def wblocks(W):  # [o,d] -> [ot, p(d%128), (d//128)*128 + o_in] f32
        a = W.astype(f32).reshape(NT, 128, NT, 128)       # (ot, o_in, t, p)
        return np.ascontiguousarray(a.transpose(0, 3, 2, 1)) \
                 .reshape(NT, 128, D)

    wkb = wblocks(Wk)
    wvb = wblocks(Wv)
    F8H = ml_dtypes.float8_e4m3
    wqb = wblocks(Wq)
    wob = np.ascontiguousarray(
        Wo.astype(f32).T.reshape(NT, 128, D))               # [ot, p(o_in), o']
    qt = np.ascontiguousarray(
        q.astype(f32).T.reshape(NT, 128, B).transpose(1, 0, 2)
    ).reshape(128, NT * B)                                  # [p, t*4+b]
    bk_t = np.ascontiguousarray(bk.astype(f32).reshape(NT, 128).T)
    bv_t = np.ascontiguousarray(bv.astype(f32).reshape(NT, 128).T)  # [128, NT]
    bq_t = np.ascontiguousarray(bq.astype(f32).reshape(NT, 128).T)
    bo8_r = np.ascontiguousarray(
        np.tile(bo.astype(f32)[None, :] / NCORES, (B, 1)))
    eye = np.eye(128, dtype=f32)
    onc = np.ones((128, 1), f32)
    onr = np.ones((1, 128), f32)
    alpha = np.asarray(alpha, f32)

    qt = qt.astype(F8H)
    shared = dict(qt=qt, bk=bk_t, bo8=bo8_r, eye=eye, onc=onc,
                  onr=onr)

    # cast histories to fp8 once, then per-core strided transpose (1-byte)
    kb = np.asarray(k_history, f32).astype(F8H)             # [T, B, D]
    vb = np.asarray(v_history, f32).astype(F8H)

    in_maps = []
    for c in range(NCORES):
        xk = np.ascontiguousarray(
            kb[c * TLOC:(c + 1) * TLOC].transpose(2, 1, 0)).reshape(D, R)
        xv = np.ascontiguousarray(
            vb[c * TLOC:(c + 1) * TLOC].transpose(2, 1, 0)).reshape(D, R)
        wkv = np.concatenate([wkb[OTC * c], wvb[OTC * c],
                              wkb[OTC * c + 1], wvb[OTC * c + 1]],
                             axis=0) * WSC
        wkv = wkv.astype(F8H)
        in_maps.append(dict(
            xk=xk, xv=xv, wkv=np.ascontiguousarray(wkv),
            wq=(np.ascontiguousarray(wqb[OTC * c:OTC * (c + 1)])
                .astype(f32) * WSC).astype(F8H),
            wo=(np.ascontiguousarray(wob[OTC * c:OTC * (c + 1)])
                * WSC).astype(F8H),
            bq=np.ascontiguousarray(bq_t[:, OTC * c:OTC * (c + 1)]),
            al=np.ascontiguousarray(
                np.repeat(alpha[OTC * c:OTC * (c + 1)], B)[None, :] * OSC),
            bv8=np.ascontiguousarray(
                np.repeat(bv_t[:, OTC * c:OTC * (c + 1)], B, axis=1)),
            **shared))
    return in_maps


_CACHE = {}


def kernel(**inputs):
    if "nc" not in _CACHE:
        _CACHE["nc"] = build_nc()
    nc = _CACHE["nc"]
    in_maps = prep_inputs(**{k: np.asarray(v) for k, v in inputs.items()})
    res = run_bass_kernel_spmd(nc, in_maps, core_ids=list(range(NCORES)))
    return np.asarray(res.results[0]["out"], dtype=np.float32).reshape(B, D)



# revision 2
# speedup vs baseline: 1.0193x; 1.0193x over previous
"""Kernelized (linear) attention on 8 TRN2 NeuronCores — v17 (final).

Single-collective design (vs v11's four):
  - Full Wk/Wv/Wo staged per core in device DRAM (no weight AllGather);
    main loop is T-data-parallel over the 8 cores as before.
  - ONE AllGather carries: per-core kv/ks stat partials ([128,64]+[128,64])
    plus that core's relu'd q-projection slice ([128,8]) — rank placement
    in the AG output gives per-core column offsets for free in the
    uniform SPMD program.
  - Every core then sums the 8 stat chunks locally, assembles the full
    q_k, and computes ITS 256-column slice of the output (16 FD=256
    matmuls against its Wo column slice); the host concatenates the 8
    slices — no ReduceScatter / output AllReduce.
  - AG payload is bf16 to halve wire/readback cost; kk intermediate is
    bf16 to cut SBUF traffic.

Layout per core c (hardcoded D=2048, H=16, T=4096, B=4, 8 cores):
  xk/xv: [D, R=2048] fp8, columns r = b*512 + t_local, t in [c*512,(c+1)*512)
  wkv:   [NT*256, D] fp8: rows [ot*256, ot*256+128) = Wk block ot,
         [+128, +256) = Wv block ot, in wblocks layout
         [p = d%128, (d//128)*128 + o_in], host-scaled by WSC
  wq:    [2, 128, D] fp8 wblocks layout for own ot = 2c+otl
  wo:    [NT, 128, D] fp8 = Wo.T[ot block rows, :] for ALL ot, x WSC
  qt:    [128, NT*B] fp8, [p = d%128, (d//128)*B + b]
  AG payload agin [128, 136] f32: cols 0:64 kv partials (col ot*B+b),
         64:128 ks partials, 128:136 own relu(q-proj) (ot-major, b-minor)
"""

import numpy as np
import ml_dtypes

from concourse import bass, bacc, mybir, tile
from concourse.bass_utils import run_bass_kernel_spmd

BF16 = ml_dtypes.bfloat16
F8H = ml_dtypes.float8_e4m3

D, H, T, B = 2048, 16, 4096, 4
HD = D // H           # 128
NCORES = 8
TLOC = T // NCORES    # 512 history rows per core
R = TLOC * B          # 2048 projection rows per core
NT = D // 128         # 16 tiles along d (contraction) and o (output)
NTT = NT // 2         # 8 DoubleRow contraction pairs
OTC = NT // NCORES    # 2 output tiles (heads) owned per core
EPS = 1e-6
F32 = mybir.dt.float32
BF = mybir.dt.bfloat16
F8 = mybir.dt.float8e4
WSC = 64.0    # host pre-scale on Wk/Wv/Wq/Wo so fp8 stays in normal range
OSC = 8192.0  # pre-scale on opre (tiny values) before fp8 cast
AF = mybir.ActivationFunctionType
OP = mybir.AluOpType
AGC = 136     # AG payload columns: 64 kv + 64 ks + 8 qk


def build_nc(K=1):
    nc = bacc.Bacc("TRN2", target_bir_lowering=False, debug=False,
                   enable_asserts=False, num_devices=NCORES)

    def din(name, shape, dt):
        return nc.dram_tensor(name, list(shape), dt, kind="ExternalInput").ap()

    xk_d = din("xk", (NTT * 128, 2 * R), F8)
    xv_d = din("xv", (NTT * 128, 2 * R), F8)
    wkv_d = din("wkv", (NT * 128, 2 * D), F8)
    wq_d = din("wq", (OTC, 128, D), F8)
    wo_d = din("wo", (NT, 128, D // NCORES), F8)
    qt_d = din("qt", (128, NT * B), F8)
    bk_d = din("bk", (128, NT), F32)
    bv64_d = din("bv64", (128, NT * B), F32)
    bq_d = din("bq", (128, OTC), F32)
    bo_d = din("bo", (B, D // NCORES), F32)
    al_d = din("al", (1, NT * B), F32)
    onc_d = din("onc", (128, 1), F32)
    onr_d = din("onr", (1, 128), F32)

    out_d = nc.dram_tensor("out", [B, D // NCORES], F32,
                           kind="ExternalOutput").ap()

    with tile.TileContext(nc) as tc:
        import contextlib
        with contextlib.ExitStack() as ctx:
            p_xk = ctx.enter_context(tc.tile_pool(name="xk", bufs=NT))
            p_xv = ctx.enter_context(tc.tile_pool(name="xv", bufs=NT))
            p_w = ctx.enter_context(tc.tile_pool(name="w", bufs=4))
            p_wo = ctx.enter_context(tc.tile_pool(name="wo", bufs=4))
            p_ep = ctx.enter_context(tc.tile_pool(name="ep", bufs=3))
            p_pr = ctx.enter_context(tc.tile_pool(name="pr", bufs=2))
            p_c1 = ctx.enter_context(tc.tile_pool(name="c1", bufs=1))
            p_c2 = ctx.enter_context(tc.tile_pool(name="c2", bufs=2))
            p_st = ctx.enter_context(tc.tile_pool(name="st", bufs=2))
            p_dr = ctx.enter_context(tc.tile_pool(name="dr", bufs=2,
                                                  space="DRAM"))
            p_mk = ctx.enter_context(
                tc.tile_pool(name="mmk", bufs=3, space="PSUM"))
            p_mv = ctx.enter_context(
                tc.tile_pool(name="mmv", bufs=3, space="PSUM"))
            p_fin = ctx.enter_context(
                tc.tile_pool(name="fin", bufs=1, space="PSUM"))
            p_op = ctx.enter_context(
                tc.tile_pool(name="opp", bufs=1, space="PSUM"))

            DRm = mybir.MatmulPerfMode.DoubleRow

            for _it in range(K):
                # ---- small resident loads ------------------------------
                qt_s = p_c1.tile([128, NT * B], F8, tag="qt")
                nc.sync.dma_start(out=qt_s[:], in_=qt_d[:, :])
                bk_s = p_c1.tile([128, NT], F32, tag="bk")
                nc.sync.dma_start(out=bk_s[:], in_=bk_d[:, :])
                bv64_s = p_c1.tile([128, NT * B], F32, tag="bv64")
                nc.sync.dma_start(out=bv64_s[:], in_=bv64_d[:, :])
                bq_s = p_c1.tile([128, OTC], F32, tag="bq")
                nc.sync.dma_start(out=bq_s[:], in_=bq_d[:, :])
                bo_s = p_c1.tile([B, D // NCORES], F32, tag="bo")
                nc.sync.dma_start(out=bo_s[:], in_=bo_d[:, :])
                al_s = p_c1.tile([1, NT * B], F32, tag="al")
                nc.sync.dma_start(out=al_s[:], in_=al_d[:, :])
                onc_s = p_c1.tile([128, 1], F32, tag="onc")
                nc.sync.dma_start(out=onc_s[:], in_=onc_d[:, :])
                onr_s = p_c1.tile([1, 128], F32, tag="onr")
                nc.sync.dma_start(out=onr_s[:], in_=onr_d[:, :])
                wq_s = []
                for ol in range(OTC):
                    w = p_c1.tile([128, D], F8, tag=f"wq{ol}")
                    nc.sync.dma_start(out=w[:], in_=wq_d[ol])
                    wq_s.append(w)


                # ---- AG staging buffer (stats accumulate into it) ------
                arst = p_c2.tile([128, AGC], F32, tag="arst")

                # ---- q projection for own 2 heads ----------------------
                qp = p_fin.tile([128, OTC * B], F32, tag="tn",
                                name=f"qp{_it}")
                for ol in range(OTC):
                    for t in range(NT):
                        nc.tensor.matmul(
                            qp[:, ol * B:(ol + 1) * B],
                            wq_s[ol][:, t * 128:(t + 1) * 128],
                            qt_s[:, t * B:(t + 1) * B],
                            start=(t == 0), stop=(t == NT - 1))
                for ol in range(OTC):
                    nc.scalar.activation(
                        arst[:, 128 + ol * B:128 + (ol + 1) * B],
                        qp[:, ol * B:(ol + 1) * B], AF.Relu,
                        bias=bq_s[:, ol:ol + 1], scale=1.0 / WSC)

                # ---- resident x loads ----------------------------------
                xk_t, xv_t = [], []
                for tt in range(NTT):
                    a = p_xk.tile([128, 2, R], F8, tag="xk")
                    nc.sync.dma_start(out=a[:],
                                      in_=xk_d[tt * 128:(tt + 1) * 128, :])
                    xk_t.append(a)
                    b_ = p_xv.tile([128, 2, R], F8, tag="xv")
                    nc.sync.dma_start(out=b_[:],
                                      in_=xv_d[tt * 128:(tt + 1) * 128, :])
                    xv_t.append(b_)

                # ---- main loop: K/V projections + fused stats ----------
                for ot in range(NT):
                    wkv_s = p_w.tile([128, 2, NTT, 2, 128], F8, tag="wkv")
                    nc.scalar.dma_start(
                        out=wkv_s[:],
                        in_=wkv_d[ot * 128:(ot + 1) * 128, :])
                    wk_s = wkv_s[:, 0]
                    wv_s = wkv_s[:, 1]
                    for b in range(B):
                        c0 = b * 512
                        ck = ot * B + b
                        kp = p_mk.tile([128, 512], F32, tag="mmk")
                        for tt in range(NTT):
                            nc.tensor.matmul(
                                kp[:], wk_s[:, tt],
                                xk_t[tt][:, :, c0:c0 + 512],
                                start=(tt == 0), stop=(tt == NTT - 1),
                                perf_mode=DRm)
                        vp = p_mv.tile([128, 512], F32, tag="mmv")
                        for tt in range(NTT):
                            nc.tensor.matmul(
                                vp[:], wv_s[:, tt],
                                xv_t[tt][:, :, c0:c0 + 512],
                                start=(tt == 0), stop=(tt == NTT - 1),
                                perf_mode=DRm)
                        kk = p_ep.tile([128, 512], BF, tag="kk")
                        nc.scalar.activation(
                            kk[:], kp[:], AF.Relu,
                            bias=bk_s[:, ot:ot + 1], scale=1.0 / WSC,
                            accum_out=arst[:, 64 + ck:64 + ck + 1])
                        pr = p_pr.tile([128, 512], BF, tag="pr")
                        nc.vector.scalar_tensor_tensor(
                            pr[:], kk[:], EPS, vp[:], OP.add, OP.mult,
                            accum_out=arst[:, ck:ck + 1])

                # ---- single AllGather: stats + own q_k -----------------
                ag_in = p_dr.tile([128, AGC], BF, tag="agin")
                nc.gpsimd.dma_start(out=ag_in[:], in_=arst[:])
                ag_out = p_dr.tile([NCORES * 128, AGC], BF, tag="agout",
                                   addr_space="Shared")
                nc.gpsimd.collective_compute(
                    "AllGather", OP.bypass,
                    replica_groups=[list(range(NCORES))],
                    ins=[ag_in.opt()], outs=[ag_out.opt()])

                # ---- local reduce of the 8 chunks ----------------------
                chunks = []
                for r in range(NCORES):
                    ch = p_st.tile([128, AGC], BF, tag=f"ch{r % 2}",
                                   name=f"ch{r}_{_it}")
                    nc.sync.dma_start(out=ch[:],
                                      in_=ag_out[r * 128:(r + 1) * 128, :])
                    chunks.append(ch)
                g = p_c2.tile([128, 128], F32, tag="g")
                nc.vector.tensor_tensor(g[:], chunks[0][:, 0:128],
                                        chunks[1][:, 0:128], OP.add)
                for r in range(2, NCORES):
                    nc.vector.tensor_tensor(g[:], g[:],
                                            chunks[r][:, 0:128], OP.add)
                qk = p_c2.tile([128, NT * B], F32, tag="qk")
                for r in range(NCORES):
                    nc.vector.tensor_copy(qk[:, r * 8:(r + 1) * 8],
                                          chunks[r][:, 128:136])

                # ---- combine stats (all 16 heads) ----------------------
                hs = p_fin.tile([1, NT * B], F32, tag="tn",
                                name=f"hs{_it}")
                nc.tensor.matmul(hs[:], onc_s[:], g[:, 64:128],
                                 start=True, stop=True)
                den = p_c2.tile([1, NT * B], F32, tag="den")
                nc.vector.tensor_scalar(den[:], hs[:], EPS * T * HD + EPS,
                                        None, OP.add)
                rden = p_c2.tile([1, NT * B], F32, tag="rden")
                nc.vector.reciprocal(rden[:], den[:])
                rr = p_c2.tile([1, NT * B], F32, tag="rr")
                nc.vector.tensor_tensor(rr[:], rden[:], al_s[:], OP.mult)
                bcr_ps = p_fin.tile([128, NT * B], F32, tag="tn",
                                    name=f"bcr{_it}")
                nc.tensor.matmul(bcr_ps[:], onr_s[:], rr[:], start=True,
                                 stop=True)
                kvb = p_st.tile([128, NT * B], F32, tag="kvb")
                nc.vector.scalar_tensor_tensor(
                    kvb[:], g[:, 64:128], T * EPS, bv64_s[:], OP.add,
                    OP.mult)
                kvc = p_st.tile([128, NT * B], F32, tag="kvc")
                nc.vector.scalar_tensor_tensor(
                    kvc[:], g[:, 0:64], 1.0 / WSC, kvb[:], OP.mult, OP.add)
                kvr = p_c2.tile([128, NT * B], F32, tag="kvr")
                nc.vector.tensor_tensor(kvr[:], kvc[:], bcr_ps[:], OP.mult)
                opre = p_c2.tile([128, NT * B], F8, tag="opre")
                nc.vector.scalar_tensor_tensor(
                    opre[:], qk[:], EPS, kvr[:], OP.add, OP.mult)

                # ---- Wo apply: own 256-column slice only ---------------
                OC = D // NCORES
                op_ps = p_op.tile([B, OC], F32, tag="opp",
                                  name=f"op_{_it}")
                for ol in range(NT):
                    w = p_wo.tile([128, OC], F8, tag="wo")
                    nc.scalar.dma_start(out=w[:], in_=wo_d[ol])
                    nc.tensor.matmul(
                        op_ps[:], opre[:, ol * B:(ol + 1) * B], w[:],
                        start=(ol == 0), stop=(ol == NT - 1))
                opart = p_st.tile([B, OC], F32, tag="opart")
                nc.vector.scalar_tensor_tensor(
                    opart[:], op_ps[:], 1.0 / (OSC * WSC),
                    bo_s[:], OP.mult, OP.add)
                nc.sync.dma_start(out=out_d[:, :], in_=opart[:])

    nc.finalize()
    from concourse import bass_interp
    nc.m = bass_interp.get_hw_module(nc.m)
    return nc


def prep_inputs(q, k_history, v_history, Wq, bq, Wk, bk, Wv, bv, Wo, bo,
                alpha):
    """Host-side sharding + layout transforms. Returns in_maps for 8 cores."""
    f32 = np.float32

    def wblocks(W):  # [o,d] -> [ot, p(d%128), (d//128)*128 + o_in] f32
        a = W.astype(f32).reshape(NT, 128, NT, 128)       # (ot, o_in, t, p)
        return np.ascontiguousarray(a.transpose(0, 3, 2, 1)) \
                 .reshape(NT, 128, D)

    wkb = wblocks(Wk)
    wvb = wblocks(Wv)
    wqb = wblocks(Wq)
    wob = np.ascontiguousarray(
        Wo.astype(f32).T.reshape(NT, 128, D))               # [ot, p(o_in), o']
    qt = np.ascontiguousarray(
        q.astype(f32).T.reshape(NT, 128, B).transpose(1, 0, 2)
    ).reshape(128, NT * B).astype(F8H)                      # [p, t*4+b]
    bk_t = np.ascontiguousarray(bk.astype(f32).reshape(NT, 128).T)
    bv_t = bv.astype(f32).reshape(NT, 128).T                # [128, NT]
    bv64 = np.ascontiguousarray(np.repeat(bv_t, B, axis=1))  # [128, NT*B]
    bq_t = np.ascontiguousarray(bq.astype(f32).reshape(NT, 128).T)
    bo_r = np.ascontiguousarray(np.tile(bo.astype(f32)[None, :], (B, 1)))
    onc = np.ones((128, 1), f32)
    onr = np.ones((1, 128), f32)
    alpha = np.asarray(alpha, f32)
    al = np.ascontiguousarray(np.repeat(alpha, B)[None, :] * OSC)

    # per-ot interleave [p][kv][tt][s][o] -> [NT*128, 2*D], scaled for fp8
    wkv = np.stack([wkb, wvb], axis=2)          # [NT, 128, 2, D]
    wkv = np.ascontiguousarray(wkv * WSC).astype(F8H).reshape(NT * 128, 2 * D)
    wo_full = np.ascontiguousarray(wob * WSC).astype(F8H)

    shared = dict(qt=qt, bk=bk_t, bv64=bv64, al=al, onc=onc,
                  onr=onr, wkv=wkv)

    kb = np.asarray(k_history, f32).astype(F8H)             # [T, B, D]
    vb = np.asarray(v_history, f32).astype(F8H)

    in_maps = []
    for c in range(NCORES):
        def xl(h):  # [TLOC,B,D] -> [NTT*128, 2*R]: [tt][p][s][b][t]
            a = h[c * TLOC:(c + 1) * TLOC].transpose(2, 1, 0)  # [D,B,TLOC]
            a = a.reshape(NTT, 2, 128, B * TLOC)               # [tt,s,p,r]
            return np.ascontiguousarray(
                a.transpose(0, 2, 1, 3)).reshape(NTT * 128, 2 * R)
        xk = xl(kb)
        xv = xl(vb)
        oc = D // NCORES
        in_maps.append(dict(
            xk=xk, xv=xv,
            wq=(np.ascontiguousarray(wqb[OTC * c:OTC * (c + 1)])
                * WSC).astype(F8H),
            bq=np.ascontiguousarray(bq_t[:, OTC * c:OTC * (c + 1)]),
            wo=np.ascontiguousarray(wo_full[:, :, c * oc:(c + 1) * oc]),
            bo=np.ascontiguousarray(bo_r[:, c * oc:(c + 1) * oc]),
            **shared))
    return in_maps


_CACHE = {}


def kernel(**inputs):
    if "nc" not in _CACHE:
        _CACHE["nc"] = build_nc(K=1)
    nc = _CACHE["nc"]
    in_maps = prep_inputs(**{k: np.asarray(v) for k, v in inputs.items()})
    res = run_bass_kernel_spmd(nc, in_maps, core_ids=list(range(NCORES)))
    return np.concatenate(
        [np.asarray(res.results[c]["out"], dtype=np.float32)
         for c in range(NCORES)], axis=1)
